# revision 14
# baseline (speedup 1.0000x reference)
"""Gaussian-KDE logsumexp kernel for Trainium2 (8 NeuronCores, SPMD).

Math: out[t] = ln( sum_n exp(a_tn) ) - Z,  a_tn = -0.5*scale_n*dist2[t,n].
One fp16 K=66 GEMM produces a_tn + K0 (the Schraudolph/exp shift K0 is folded
into the train-side augmented row):
    xhat[:, t] = [test_t (64), test_sq_t, 1]
    yhat[:, n] = [scale_n*train_n (64), -.5*scale_n, -.5*scale_n*train_sq_n + K0]
Per chunk the exp+reduce is split across engines:
  - ACT: exp table on cols [0:A) with bias -K0, free-dim accumulation.
  - DVE: Schraudolph fast exp on cols [A:2048): i32 = max(psum*C1, 0) is
    the float bit pattern of 2^((a+K0)/ln2 - 127) ~= exp(a); the bitcast-f32
    view is then reduce-summed.
Host does the final cross-core sum, ln, and -Z.

Sharding: 4 test-quarters x 2 train-halves = 8 cores; each core gets
512 test points (4 t-tiles of 128) x 2048 train points, all-fp16 inputs
(337 KB/core) DMA'd as one packed-f32 tensor split over three queues
(Scalar + Sync HWDGE, Pool SWDGE). Partial sums [128, 4]+[128, 4] go back
over Scalar/Sync in parallel; the per-engine exit drains fence the acks.
"""

import math
from contextlib import ExitStack

import numpy as np

import concourse.bacc as bacc
import concourse.mybir as mybir
from concourse.bass_utils import run_bass_kernel_spmd

N_CORES = 8
NT, NTR, D = 2048, 4096, 64
KA = D + 2                   # 66: augmented contraction dim
TW, TRW = 4, 2               # test ways x train ways (TW*TRW == 8)
TPC = NT // TW               # 512 test points per core
NPC = NTR // TRW             # 2048 train points per core
P = 128                      # partition tile of test points
T_TILES = TPC // P           # 4
MM_N = 512                   # matmul free-dim (one PSUM bank, fp32)
F32 = mybir.dt.float32
F16 = mybir.dt.float16
I32 = mybir.dt.int32
I16 = mybir.dt.int16
BF16 = mybir.dt.bfloat16

Z_CONST = float(0.5 * D * math.log(2.0 * math.pi) + math.log(NTR))  # h = 1
C1 = float(2.0 ** 23 / math.log(2.0))            # Schraudolph scale (fp32)
C1B = float(2.0 ** 7 / math.log(2.0))            # Schraudolph scale (bf16)
K0 = float((127.0 - 0.0434609) * math.log(2.0))  # exp-arg shift (~88.0)

A_ACT = 1344                 # ACT exp columns per 2048-col chunk (rest: DVE)

# packed-f32 column counts of the combined [x | y] input tensor
XC = TPC // 2                # 256
YC = NPC // 2                # 1024


def build_program(impl="dve16", a_act=A_ACT, warmup=0):
    """impl: 'act' (ACT does all exp), 'dve' (ACT+DVE split, fp32-bit
    Schraudolph), 'dve16' (bf16-bit Schraudolph; 2-byte lanes let the DVE
    reduce run in 2x mode)."""
    split = impl != "act"
    A = a_act if split else NPC
    B = NPC - A
    bits16 = impl == "dve16"
    ei_dt = I16 if bits16 else I32
    ei_ft = BF16 if bits16 else F32
    c1 = C1B if bits16 else C1

    nc = bacc.Bacc("TRN2")
    xy = nc.declare_dram_parameter("xy", [KA, XC + YC], F32, isOutput=False)
    # two separate contiguous outputs: concurrent DMAs must never share a
    # 64B DRAM line (partial-line writes from different DMA engines race)
    out_a = nc.declare_dram_parameter("out_a", [P, T_TILES], F32, isOutput=True)
    out_b = nc.declare_dram_parameter("out_b", [P, T_TILES], F32, isOutput=True)

    with ExitStack() as ctx:
        sb = lambda nm, shape, dt: ctx.enter_context(nc.sbuf_tensor(nm, shape, dt))
        comb = sb("comb", [KA, XC + YC], F32)
        xs = comb[:, 0:XC].bitcast(F16)              # [66, 512]
        ys = comb[:, XC:XC + YC].bitcast(F16)        # [66, 2048]
        et = [sb(f"et{k}", [P, A], F32) for k in range(2)]
        ei = [sb(f"ei{k}", [P, max(B, 1)], ei_dt) for k in range(2)]
        dummy_in = sb("dummy_in", [P, 1], F32)
        dummy_out = sb("dummy_out", [P, 1], F32)
        negk0 = sb("negk0", [P, 1], F32)
        sums = sb("sums", [P, 2 * T_TILES], F32)
        pt = [
            ctx.enter_context(nc.psum_tensor(f"pt{k}", [P, NPC], F32))
            for k in range(2)
        ]

        sxd = ctx.enter_context(nc.semaphore("sxd"))
        syq = [ctx.enter_context(nc.semaphore(f"syq{q}")) for q in range(4)]
        spe = ctx.enter_context(nc.semaphore("spe"))
        spej = ctx.enter_context(nc.semaphore("spej"))  # fires after j2: ACT cols ready
        sact = ctx.enter_context(nc.semaphore("sact"))
        sdve = ctx.enter_context(nc.semaphore("sdve"))
        svz = ctx.enter_context(nc.semaphore("svz"))
        so = ctx.enter_context(nc.semaphore("so"))  # out-DMA acks (engine drain fences them)

        # --- Scalar (ACT, HWDGE): x then y quarter 3
        nc.scalar.dma_start(out=comb[:, 0:XC], in_=xy[:, 0:XC]).then_inc(sxd, 16)
        for q in (3,):
            lo, hi = XC + q * 256, XC + (q + 1) * 256
            nc.scalar.dma_start(out=comb[:, lo:hi], in_=xy[:, lo:hi]).then_inc(
                syq[q], 16)
        # --- Pool (software DGE, otherwise idle): y quarter 1
        for q in (1,):
            lo, hi = XC + q * 256, XC + (q + 1) * 256
            nc.gpsimd.dma_start(out=comb[:, lo:hi], in_=xy[:, lo:hi]).then_inc(
                syq[q], 16)
        # dummy exp triggers the activation-table load at boot; bias AP is
        # garbage at this point, output unused
        nc.scalar.activation(
            dummy_out[:], dummy_in[:], mybir.ActivationFunctionType.Exp,
            bias=negk0[:],
        )

        # --- Sync (HWDGE): y quarters 0, 2
        for q in (0, 2):
            lo, hi = XC + q * 256, XC + (q + 1) * 256
            nc.sync.dma_start(out=comb[:, lo:hi], in_=xy[:, lo:hi]).then_inc(
                syq[q], 16)

        # --- DVE: constant
        nc.vector.memset(negk0[:], -K0).then_inc(svz, 1)

        # --- PE: optional p-state warmup on garbage, then the real stream
        for w in range(warmup):
            nc.tensor.matmul(
                pt[1][:, (w % 4) * MM_N:(w % 4 + 1) * MM_N],
                xs[:, 0:P],
                ys[:, 0:MM_N],
                start=True,
                stop=True,
            )
        for k in range(T_TILES):
            for j in range(NPC // MM_N):
                if k == 0 and j == 0:
                    nc.tensor.wait_ge(sxd, 16)
                if k == 0:
                    nc.tensor.wait_ge(syq[j], 16)
                if k >= 2 and j == 0:
                    nc.tensor.wait_ge(sact, k - 1)
                    if split:
                        nc.tensor.wait_ge(sdve, k - 1)
                mm = nc.tensor.matmul(
                    pt[k % 2][:, j * MM_N:(j + 1) * MM_N],
                    xs[:, k * P:(k + 1) * P],
                    ys[:, j * MM_N:(j + 1) * MM_N],
                    start=True,
                    stop=True,
                )
                if j == 2:
                    mm.then_inc(spej, 1)
            mm.then_inc(spe, 1)

        # --- ACT: exp + accumulate per chunk, then its half of the output
        nc.scalar.wait_ge(svz, 1)
        assert A <= 3 * MM_N, "ACT share must be covered by matmuls j0..j2"
        for k in range(T_TILES):
            nc.scalar.wait_ge(spej, k + 1)
            nc.scalar.activation(
                out=et[k % 2][:],
                in_=pt[k % 2][:, 0:A],
                func=mybir.ActivationFunctionType.Exp,
                bias=negk0[:],
                accum_out=sums[:, k:k + 1],
            ).then_inc(sact, 1)
        # sact rides on the auto-emitted READ_ACCUMULATOR; without this wait
        # the DMA descriptor posts before the final accum lands in SBUF.
        # Issued from Scalar so it runs in parallel with Sync's out_b issue
        # (the exit epilogue only starts once every engine reaches barrier1).
        nc.scalar.wait_ge(sact, T_TILES)
        nc.scalar.dma_start(
            out=out_a[:], in_=sums[:, 0:T_TILES]
        ).then_inc(so, 16)

        if split:
            # --- DVE: Schraudolph fast-exp + reduce of its own share
            for k in range(T_TILES):
                nc.vector.wait_ge(spe, k + 1)
                nc.vector.tensor_scalar(
                    out=ei[k % 2][:],
                    in0=pt[k % 2][:, A:NPC],
                    scalar1=c1,
                    scalar2=0.0,
                    op0=mybir.AluOpType.mult,
                    op1=mybir.AluOpType.max,
                ).then_inc(sdve, 1)
                r = nc.vector.reduce_sum(
                    out=sums[:, T_TILES + k:T_TILES + k + 1],
                    in_=ei[k % 2][:].bitcast(ei_ft),
                    axis=mybir.AxisListType.X,
                )
            r.then_inc(sdve, 16)  # final value: T_TILES + 16

            # --- Sync ships the split half of the sums
            nc.sync.wait_ge(sdve, T_TILES + 16)
            nc.sync.dma_start(
                out=out_b[:],
                in_=sums[:, T_TILES:2 * T_TILES],
            ).then_inc(so, 16)

    nc.compile()
    _strip_boot_barrier(nc)
    return nc


def _strip_boot_barrier(nc):
    """Drop the framework's all-engine boot barrier and const-AP memsets so
    every engine starts issuing immediately, and push the hoisted activation
    table load back behind the input-DMA issues (it only has to precede the
    dummy activation)."""
    blk = nc.main_func.blocks[0]
    insts = list(blk.instructions)
    drop = set()
    for i, inst in enumerate(insts):
        tn = type(inst).__name__
        if tn == "InstEventSemaphore" and inst.name.startswith("barrier_"):
            drop.add(inst.name)
            if i > 0 and type(insts[i - 1]).__name__ == "InstDrain":
                drop.add(insts[i - 1].name)
        elif tn == "InstMemset" and inst.outs and "const-" in str(inst.outs[0]):
            drop.add(inst.name)
    insts = [i for i in insts if i.name not in drop]
    loads = [i for i in insts if type(i).__name__ == "InstLoadActFuncSet"]
    if loads:
        assert len(loads) == 1, [l.name for l in loads]
        load = loads[0]
        rest = [i for i in insts if i is not load]
        first_act = next(
            k for k, i in enumerate(rest)
            if type(i).__name__ == "InstActivation"
        )
        insts = rest[:first_act] + [load] + rest[first_act:]
    blk.instructions[:] = insts


_PROG = {}


def _get_prog(impl="dve16", a_act=A_ACT, warmup=0):
    key = (impl, a_act, warmup)
    if key not in _PROG:
        _PROG[key] = build_program(impl, a_act, warmup)
    return _PROG[key]


def _prepare(test_Xs, train_Xs, weights):
    test_Xs = np.asarray(test_Xs, dtype=np.float32)
    train_Xs = np.asarray(train_Xs, dtype=np.float32)
    weights = np.asarray(weights, dtype=np.float32)

    test_sq = (test_Xs.astype(np.float64) ** 2).sum(1)
    train_sq = (train_Xs.astype(np.float64) ** 2).sum(1)
    scale = weights.astype(np.float64) ** 2

    xhat = np.empty((KA, NT), np.float16)
    xhat[:D] = test_Xs.T
    xhat[D] = test_sq
    xhat[D + 1] = 1.0

    yhat = np.empty((KA, NTR), np.float16)
    yhat[:D] = (train_Xs.astype(np.float64) * scale[:, None]).T
    yhat[D] = -0.5 * scale
    yhat[D + 1] = -0.5 * scale * train_sq + K0
    return xhat, yhat


def kernel(test_Xs, train_Xs, weights, impl="dve16", a_act=A_ACT, warmup=0,
           trace=False):
    xhat, yhat = _prepare(test_Xs, train_Xs, weights)
    nc = _get_prog(impl, a_act, warmup)
    in_maps = []
    for c in range(N_CORES):
        i, j = c >> 1, c & 1
        xy = np.empty((KA, 2 * (XC + YC)), np.float16)
        xy[:, 0:TPC] = xhat[:, i * TPC:(i + 1) * TPC]
        xy[:, TPC:] = yhat[:, j * NPC:(j + 1) * NPC]
        in_maps.append({"xy": xy.view(np.float32)})
    res = run_bass_kernel_spmd(nc, in_maps, list(range(N_CORES)), trace=trace)

    S = np.zeros(NT, np.float64)
    for c in range(N_CORES):
        i = c >> 1
        part = res.results[c]["out_a"].astype(np.float64)    # [128, 4]
        if impl != "act":
            part = part + res.results[c]["out_b"].astype(np.float64)
        # t = i*TPC + k*P + p  <-> column-major flatten of part[p, k]
        S[i * TPC:(i + 1) * TPC] += part.T.ravel()
    out = (np.log(S) - Z_CONST).astype(np.float32)
    if trace:
        kernel.last_results = res
    return out


# revision 15
# speedup vs baseline: 1.1261x; 1.1261x over previous
"""Gaussian-KDE logsumexp kernel for Trainium2 (8 NeuronCores, SPMD).

Math: out[t] = ln( sum_n exp(a_tn) ) - Z,  a_tn = -0.5*scale_n*dist2[t,n].
One fp16 K=66 GEMM produces a_tn + K0 (the Schraudolph/exp shift K0 is folded
into the train-side augmented row):
    xhat[:, t] = [test_t (64), test_sq_t, 1]
    yhat[:, n] = [scale_n*train_n (64), -.5*scale_n, -.5*scale_n*train_sq_n + K0]
Per chunk the exp+reduce is split across engines:
  - ACT: exp table on cols [0:A) with bias -K0, free-dim accumulation.
  - DVE: Schraudolph fast exp on cols [A:2048): i32 = max(psum*C1, 0) is
    the float bit pattern of 2^((a+K0)/ln2 - 127) ~= exp(a); the bitcast-f32
    view is then reduce-summed.
Host does the final cross-core sum, ln, and -Z.

Sharding: 4 test-quarters x 2 train-halves = 8 cores; each core gets
512 test points (4 t-tiles of 128) x 2048 train points, all-fp16 inputs
(337 KB/core) DMA'd as one packed-f32 tensor split over three queues
(Scalar + Sync HWDGE, Pool SWDGE). Partial sums [128, 4]+[128, 4] go back
over Scalar/Sync in parallel; the per-engine exit drains fence the acks.
"""

import math
from contextlib import ExitStack

import numpy as np

import concourse.bacc as bacc
import concourse.mybir as mybir
from concourse.bass_utils import run_bass_kernel_spmd

N_CORES = 8
NT, NTR, D = 2048, 4096, 64
KA = D + 2                   # 66: augmented contraction dim
TW, TRW = 4, 2               # test ways x train ways (TW*TRW == 8)
TPC = NT // TW               # 512 test points per core
NPC = NTR // TRW             # 2048 train points per core
P = 128                      # partition tile of test points
T_TILES = TPC // P           # 4
MM_N = 512                   # matmul free-dim (one PSUM bank, fp32)
F32 = mybir.dt.float32
F16 = mybir.dt.float16
I32 = mybir.dt.int32
I16 = mybir.dt.int16
BF16 = mybir.dt.bfloat16

Z_CONST = float(0.5 * D * math.log(2.0 * math.pi) + math.log(NTR))  # h = 1
C1 = float(2.0 ** 23 / math.log(2.0))            # Schraudolph scale (fp32)
C1B = float(2.0 ** 7 / math.log(2.0))            # Schraudolph scale (bf16)
K0 = float((127.0 - 0.0434609) * math.log(2.0))  # exp-arg shift (~88.0)

A_ACT = 1344                 # ACT exp columns per 2048-col chunk (rest: DVE)

# packed-f32 column counts of the combined [x | y] input tensor
XC = TPC // 2                # 256
YC = NPC // 2                # 1024


def build_program(impl="dve16", a_act=A_ACT, warmup=0):
    """impl: 'act' (ACT does all exp), 'dve' (ACT+DVE split, fp32-bit
    Schraudolph), 'dve16' (bf16-bit Schraudolph; 2-byte lanes let the DVE
    reduce run in 2x mode)."""
    split = impl != "act"
    A = a_act if split else NPC
    B = NPC - A
    bits16 = impl == "dve16"
    ei_dt = I16 if bits16 else I32
    ei_ft = BF16 if bits16 else F32
    c1 = C1B if bits16 else C1

    nc = bacc.Bacc("TRN2")
    xy = nc.declare_dram_parameter("xy", [KA, XC + YC], F32, isOutput=False)
    # two separate contiguous outputs: concurrent DMAs must never share a
    # 64B DRAM line (partial-line writes from different DMA engines race)
    out_a = nc.declare_dram_parameter("out_a", [P, T_TILES], F32, isOutput=True)
    out_b = nc.declare_dram_parameter("out_b", [P, T_TILES], F32, isOutput=True)

    with ExitStack() as ctx:
        sb = lambda nm, shape, dt: ctx.enter_context(nc.sbuf_tensor(nm, shape, dt))
        comb = sb("comb", [KA, XC + YC], F32)
        xs = comb[:, 0:XC].bitcast(F16)              # [66, 512]
        ys = comb[:, XC:XC + YC].bitcast(F16)        # [66, 2048]
        et = [sb(f"et{k}", [P, A], F32) for k in range(2)]
        ei = [sb(f"ei{k}", [P, max(B, 1)], ei_dt) for k in range(2)]
        dummy_in = sb("dummy_in", [P, 1], F32)
        dummy_out = sb("dummy_out", [P, 1], F32)
        negk0 = sb("negk0", [P, 1], F32)
        sums = sb("sums", [P, 2 * T_TILES], F32)
        pt = [
            ctx.enter_context(nc.psum_tensor(f"pt{k}", [P, NPC], F32))
            for k in range(2)
        ]

        sxd = ctx.enter_context(nc.semaphore("sxd"))
        syq = [ctx.enter_context(nc.semaphore(f"syq{q}")) for q in range(4)]
        spe = ctx.enter_context(nc.semaphore("spe"))
        sact = ctx.enter_context(nc.semaphore("sact"))
        sdve = ctx.enter_context(nc.semaphore("sdve"))
        svz = ctx.enter_context(nc.semaphore("svz"))
        so = ctx.enter_context(nc.semaphore("so"))  # out-DMA acks (engine drain fences them)

        # --- Scalar (ACT, HWDGE): x then y quarter 3
        nc.scalar.dma_start(out=comb[:, 0:XC], in_=xy[:, 0:XC]).then_inc(sxd, 16)
        for q in (3,):
            lo, hi = XC + q * 256, XC + (q + 1) * 256
            nc.scalar.dma_start(out=comb[:, lo:hi], in_=xy[:, lo:hi]).then_inc(
                syq[q], 16)
        # --- Pool (software DGE, otherwise idle): y quarter 1
        for q in (1,):
            lo, hi = XC + q * 256, XC + (q + 1) * 256
            nc.gpsimd.dma_start(out=comb[:, lo:hi], in_=xy[:, lo:hi]).then_inc(
                syq[q], 16)
        # dummy exp triggers the activation-table load at boot; bias AP is
        # garbage at this point, output unused
        nc.scalar.activation(
            dummy_out[:], dummy_in[:], mybir.ActivationFunctionType.Exp,
            bias=negk0[:],
        )

        # --- Sync (HWDGE): y quarters 0, 2
        for q in (0, 2):
            lo, hi = XC + q * 256, XC + (q + 1) * 256
            nc.sync.dma_start(out=comb[:, lo:hi], in_=xy[:, lo:hi]).then_inc(
                syq[q], 16)

        # --- DVE: constant
        nc.vector.memset(negk0[:], -K0).then_inc(svz, 1)

        # --- PE: optional p-state warmup on garbage, then the real stream
        for w in range(warmup):
            nc.tensor.matmul(
                pt[1][:, (w % 4) * MM_N:(w % 4 + 1) * MM_N],
                xs[:, 0:P],
                ys[:, 0:MM_N],
                start=True,
                stop=True,
            )
        for k in range(T_TILES):
            for j in range(NPC // MM_N):
                if k == 0 and j == 0:
                    nc.tensor.wait_ge(sxd, 16)
                if k == 0:
                    nc.tensor.wait_ge(syq[j], 16)
                if k >= 2 and j == 0:
                    nc.tensor.wait_ge(sact, k - 1)
                    if split:
                        nc.tensor.wait_ge(sdve, k - 1)
                mm = nc.tensor.matmul(
                    pt[k % 2][:, j * MM_N:(j + 1) * MM_N],
                    xs[:, k * P:(k + 1) * P],
                    ys[:, j * MM_N:(j + 1) * MM_N],
                    start=True,
                    stop=True,
                )
            mm.then_inc(spe, 1)

        # --- ACT: exp + accumulate per chunk, then its half of the output
        nc.scalar.wait_ge(svz, 1)
        for k in range(T_TILES):
            nc.scalar.wait_ge(spe, k + 1)
            nc.scalar.activation(
                out=et[k % 2][:],
                in_=pt[k % 2][:, 0:A],
                func=mybir.ActivationFunctionType.Exp,
                bias=negk0[:],
                accum_out=sums[:, k:k + 1],
            ).then_inc(sact, 1)
        # sact rides on the auto-emitted READ_ACCUMULATOR; without this wait
        # the DMA descriptor posts before the final accum lands in SBUF.
        # Issued from Scalar so it runs in parallel with Sync's out_b issue
        # (the exit epilogue only starts once every engine reaches barrier1).
        nc.scalar.wait_ge(sact, T_TILES)
        nc.scalar.dma_start(
            out=out_a[:], in_=sums[:, 0:T_TILES]
        ).then_inc(so, 16)

        if split:
            # --- DVE: Schraudolph fast-exp + reduce of its own share
            for k in range(T_TILES):
                nc.vector.wait_ge(spe, k + 1)
                nc.vector.tensor_scalar(
                    out=ei[k % 2][:],
                    in0=pt[k % 2][:, A:NPC],
                    scalar1=c1,
                    scalar2=0.0,
                    op0=mybir.AluOpType.mult,
                    op1=mybir.AluOpType.max,
                ).then_inc(sdve, 1)
                r = nc.vector.reduce_sum(
                    out=sums[:, T_TILES + k:T_TILES + k + 1],
                    in_=ei[k % 2][:].bitcast(ei_ft),
                    axis=mybir.AxisListType.X,
                )
            r.then_inc(sdve, 16)  # final value: T_TILES + 16

            # --- Sync ships the split half of the sums
            nc.sync.wait_ge(sdve, T_TILES + 16)
            nc.sync.dma_start(
                out=out_b[:],
                in_=sums[:, T_TILES:2 * T_TILES],
            ).then_inc(so, 16)

    nc.compile()
    _strip_boot_barrier(nc)
    return nc


def _strip_boot_barrier(nc):
    """Drop the framework's all-engine boot barrier and const-AP memsets so
    every engine starts issuing immediately, and push the hoisted activation
    table load back behind the input-DMA issues (it only has to precede the
    dummy activation)."""
    blk = nc.main_func.blocks[0]
    insts = list(blk.instructions)
    drop = set()
    for i, inst in enumerate(insts):
        tn = type(inst).__name__
        if tn == "InstEventSemaphore" and inst.name.startswith("barrier_"):
            drop.add(inst.name)
            if i > 0 and type(insts[i - 1]).__name__ == "InstDrain":
                drop.add(insts[i - 1].name)
        elif tn == "InstMemset" and inst.outs and "const-" in str(inst.outs[0]):
            drop.add(inst.name)
    insts = [i for i in insts if i.name not in drop]
    loads = [i for i in insts if type(i).__name__ == "InstLoadActFuncSet"]
    if loads:
        assert len(loads) == 1, [l.name for l in loads]
        load = loads[0]
        rest = [i for i in insts if i is not load]
        first_act = next(
            k for k, i in enumerate(rest)
            if type(i).__name__ == "InstActivation"
        )
        insts = rest[:first_act] + [load] + rest[first_act:]
    blk.instructions[:] = insts


_PROG = {}


def _get_prog(impl="dve16", a_act=A_ACT, warmup=0):
    key = (impl, a_act, warmup)
    if key not in _PROG:
        _PROG[key] = build_program(impl, a_act, warmup)
    return _PROG[key]


def _prepare(test_Xs, train_Xs, weights):
    test_Xs = np.asarray(test_Xs, dtype=np.float32)
    train_Xs = np.asarray(train_Xs, dtype=np.float32)
    weights = np.asarray(weights, dtype=np.float32)

    test_sq = (test_Xs.astype(np.float64) ** 2).sum(1)
    train_sq = (train_Xs.astype(np.float64) ** 2).sum(1)
    scale = weights.astype(np.float64) ** 2

    xhat = np.empty((KA, NT), np.float16)
    xhat[:D] = test_Xs.T
    xhat[D] = test_sq
    xhat[D + 1] = 1.0

    yhat = np.empty((KA, NTR), np.float16)
    yhat[:D] = (train_Xs.astype(np.float64) * scale[:, None]).T
    yhat[D] = -0.5 * scale
    yhat[D + 1] = -0.5 * scale * train_sq + K0
    return xhat, yhat


def kernel(test_Xs, train_Xs, weights, impl="dve16", a_act=A_ACT, warmup=0,
           trace=False):
    xhat, yhat = _prepare(test_Xs, train_Xs, weights)
    nc = _get_prog(impl, a_act, warmup)
    in_maps = []
    for c in range(N_CORES):
        i, j = c >> 1, c & 1
        xy = np.empty((KA, 2 * (XC + YC)), np.float16)
        xy[:, 0:TPC] = xhat[:, i * TPC:(i + 1) * TPC]
        xy[:, TPC:] = yhat[:, j * NPC:(j + 1) * NPC]
        in_maps.append({"xy": xy.view(np.float32)})
    res = run_bass_kernel_spmd(nc, in_maps, list(range(N_CORES)), trace=trace)

    S = np.zeros(NT, np.float64)
    for c in range(N_CORES):
        i = c >> 1
        part = res.results[c]["out_a"].astype(np.float64)    # [128, 4]
        if impl != "act":
            part = part + res.results[c]["out_b"].astype(np.float64)
        # t = i*TPC + k*P + p  <-> column-major flatten of part[p, k]
        S[i * TPC:(i + 1) * TPC] += part.T.ravel()
    out = (np.log(S) - Z_CONST).astype(np.float32)
    if trace:
        kernel.last_results = res
    return out
